# revision 25
# baseline (speedup 1.0000x reference)
"""AVWGCN (adaptive-embedding graph conv) Trainium2 Bass kernel.

Math (reference):
    A   = softmax(relu(E E^T), axis=1)            # [N, N], E: [N, D]
    T0  = I, T1 = A, T2 = 2 A A - I               # Chebyshev supports
    W   = einsum('nd,dkio->nkio', E, Wp)          # per-node weights
    b   = E @ bp                                  # per-node bias
    x_g = einsum('knm,bmc->bnkc', T, x)
    out = einsum('bnki,nkio->bno', x_g, W) + b

Restructuring (algebraically exact up to bf16 rounding):
  * Z := exp(relu(E E^T)) = max(exp(E E^T), 1) is SYMMETRIC; with row
    sums s, A = Z/s. Z stays RESIDENT IN SBUF (64KB/partition) — no
    DRAM round trip. Row sums come for free from the DVE max pass via
    accum_out.
  * y1 = A @ X, u2' = 2 A y1 (so y2 = u2' - X); the "- X" is folded
    into the weights: W_eff[k0] = Wp[k0] - Wp[k2] applied to x, Wp[k2]
    applied to u2'.
  * Final stage is node-parallel: y1^T and u2'^T (columns (b, n)) are
    exchanged via two AllToAll collectives so each core holds all 64
    batches for its 256 nodes. x^T for this core's nodes is provided
    directly as a per-core host-prepared input (no exchange needed).
    Per-node weights W[n] = sum_d E[n,d] Wp_eff[d] (+ bias row, paired
    with the ones row of R) are built on the PE while AllToAll #2
    flies; each node's output is two small accumulating matmuls.
  * Host pre-transposes/casts inputs per core (x batch-shard in
    node-major bf16, x node-shard channel-major bf16, E^T, weight
    pools in PE-ready layout) — device does zero layout work on x/E/W
    inputs.

Sharding: batch-parallel aggregation (8 cores x 8 batches),
node-parallel final stage (8 cores x 256 nodes) with AllToAll
redistribution. All matmul operands are bf16 (PSUM stays fp32).
"""

import os
import sys
import threading

sys.path.insert(0, "/opt/trn_rl_repo")

import numpy as np

import concourse.bass as bass  # noqa: E402
import concourse.mybir as mybir  # noqa: E402
from concourse import bacc  # noqa: E402
from concourse.tile import TileContext  # noqa: E402
from concourse.masks import make_identity  # noqa: E402
from concourse.bass_utils import run_bass_kernel_spmd  # noqa: E402

F32 = mybir.dt.float32
BF = mybir.dt.bfloat16
AF = mybir.ActivationFunctionType
OP = mybir.AluOpType

NCORES = 8
B = 64
BSH = B // NCORES          # 8 batches per core
N = 2048
C = 64                     # C_IN == C_OUT
D = 16                     # embedding dim
K = 3                      # Chebyshev order
P = 128                    # partitions
NT = N // P                # 16 node blocks
BC = BSH * C               # 512 = per-core (b, c) width
KI = K * C                 # 192 contraction for the final stage
NSH = N // NCORES          # 256 nodes per core in the final stage
KR = KI - P + 1            # 65: rows of the second R tile (y2 + ones)
CH = NCORES * BSH * NSH // N  # unused; keep linter quiet

T1 = 4                     # pass-1 y1 blocks (PSUM: 4 u1 banks + 4 zt)
T2 = 6                     # pass-2 y1 blocks (+2 banks for transposes)
T3 = 6                     # pass-3 y1 blocks (+2 banks for transposes)


def build_program():
    nc = bacc.Bacc("TRN2", target_bir_lowering=False, debug=False,
                   num_devices=NCORES)

    # Host-prepared per-core inputs (bf16, PE-ready layouts).
    xt = nc.dram_tensor("xt", [NT, P, BC], BF, kind="ExternalInput")
    xnt = nc.dram_tensor("xnt", [C, B * NSH], BF, kind="ExternalInput")
    et_in = nc.dram_tensor("et", [D, N], BF, kind="ExternalInput")
    etr_in = nc.dram_tensor("etr", [D, NSH], BF, kind="ExternalInput")
    wk01_in = nc.dram_tensor("wk01", [D, P * C], BF, kind="ExternalInput")
    wk2_in = nc.dram_tensor("wk2", [D, (C + 1) * C], BF,
                            kind="ExternalInput")
    # node-sharded output: this core's NSH nodes, all B batches
    out_d = nc.dram_tensor("out", [B, NSH, C], F32, kind="ExternalOutput")
    GROUP = [list(range(NCORES))]

    with TileContext(nc) as tc:
        with tc.tile_pool(name="persist", bufs=1) as pp, \
             tc.tile_pool(name="ccd", bufs=1, space="DRAM") as ccd:
            ident = pp.tile([P, P], F32)
            make_identity(nc, ident[:])
            identb = pp.tile([P, P], BF)
            make_identity(nc, identb[:])

            # DRAM bounce tensors for the collectives.
            r0send = ccd.tile([NCORES, C, BSH * NSH], BF)
            r0recv = ccd.tile([NCORES, C, BSH * NSH], BF)
            r1send = ccd.tile([NCORES, KR, BSH * NSH], BF)
            r1recv = ccd.tile([NCORES, KR, BSH * NSH], BF)

            # s bookkeeping: per-chunk partial sums, then s and 1/s.
            s_parts = pp.tile([P, NT * 2], F32)
            s_all = pp.tile([P, 2 * NT], F32)
            s_sb = s_all[:, 0:NT]
            sinv = s_all[:, NT:2 * NT]
            srep = pp.tile([P, N], F32)   # 2/s[n] replicated on all rows

            ones_r = pp.tile([1, BSH * NSH], BF)
            nc.vector.memset(ones_r[:], 1.0)

            # rr0/rr1: R columns for ALL 64 batches x my NSH nodes,
            # cols = (b_global, n_local) = (src*8 + b_local, n).
            rr0 = pp.tile([P, B * NSH], BF)
            rr1 = pp.tile([KR, B * NSH], BF)
            # x^T rows of R0 come straight from the host (this core's
            # node shard) — off the critical path, no exchange.
            for bg in range(8):
                nc.gpsimd.dma_start(
                    out=rr0[0:C, bg * BSH * NSH:(bg + 1) * BSH * NSH],
                    in_=xnt[:, bg * BSH * NSH:(bg + 1) * BSH * NSH])
            # ones row of r1send is constant
            for j in range(NCORES):
                nc.gpsimd.dma_start(out=r1send[j, C:C + 1, :],
                                    in_=ones_r[:])

            with tc.tile_pool(name="pz", bufs=1) as pz:
                z = pz.tile([P, NT * N], BF)    # Z resident: 64KB/part
                y1 = pz.tile([P, NT * BC], BF)

                with tc.tile_pool(name="p1", bufs=1) as p1:
                    xs = p1.tile([P, NT * BC], BF)
                    etsb = p1.tile([D, N], BF)
                    nc.sync.dma_start(out=etsb[:], in_=et_in[:, :])
                    for m in range(NT):
                        nc.sync.dma_start(out=xs[:, m * BC:(m + 1) * BC],
                                          in_=xt[m, :, :])

                    # ==== fused phase B + C pass 1 (t = 0..T1-1) ====
                    # Per m: PE emits Z row-block m (4x512-col matmuls,
                    # contraction D=16), scalar exps it to SBUF in 1024-col
                    # chunks, DVE applies max(.,1) in place and emits the
                    # row-sum partials via accum_out. u1 accumulation for
                    # block m-2 keeps the PE busy while exp(m) runs.
                    with tc.tile_pool(name="psz", bufs=2,
                                      space="PSUM") as psz, \
                         tc.tile_pool(name="ps1", bufs=1,
                                      space="PSUM") as ps1:
                        u1 = {t: ps1.tile([P, BC], F32, name=f"u1_{t}")
                              for t in range(T1)}

                        def u1_acc(m):
                            for t in range(T1):
                                nc.tensor.matmul(
                                    u1[t][:],
                                    z[:, m * N + t * P:m * N + (t + 1) * P],
                                    xs[:, m * BC:(m + 1) * BC],
                                    start=(m == 0), stop=(m == NT - 1))

                        for m in range(NT):
                            for jj in range(2):
                                zt = psz.tile([P, 1024], F32)
                                for q in range(2):
                                    j = 2 * jj + q
                                    nc.tensor.matmul(
                                        zt[:, q * 512:(q + 1) * 512],
                                        etsb[:, m * P:(m + 1) * P],
                                        etsb[:, j * 512:(j + 1) * 512],
                                        start=True, stop=True)
                                # exp (scalar) then max on the otherwise-
                                # idle Pool engine (u1 critical chain);
                                # row sums on DVE, off-chain.
                                zc = z[:, m * N + jj * 1024:
                                       m * N + (jj + 1) * 1024]
                                nc.scalar.activation(zc, zt[:], AF.Exp)
                                nc.gpsimd.tensor_scalar_max(zc, zc, 1.0)
                                nc.vector.reduce_sum(
                                    out=s_parts[:, 2 * m + jj:
                                                2 * m + jj + 1],
                                    in_=zc,
                                    axis=mybir.AxisListType.X)
                            if m >= 2:
                                u1_acc(m - 2)
                        u1_acc(NT - 2)
                        u1_acc(NT - 1)

                        # s = row sums of Z (pair-sum of chunk partials),
                        # then 1/s; drains of u1 wait on sinv.
                        nc.vector.tensor_tensor(
                            out=s_sb,
                            in0=s_parts[:].rearrange(
                                "p (m j) -> p m j", j=2)[:, :, 0],
                            in1=s_parts[:].rearrange(
                                "p (m j) -> p m j", j=2)[:, :, 1],
                            op=OP.add)
                        nc.vector.reciprocal(sinv, s_sb)
                        for t in range(T1):
                            nc.vector.tensor_scalar_mul(
                                y1[:, t * BC:(t + 1) * BC],
                                u1[t][:], sinv[:, t:t + 1])

                    # srep: 2/s[n] replicated on all rows (for u2' col
                    # scaling). Built during pass 2 (off PE critical path
                    # except one tiny transpose).
                    with tc.tile_pool(name="pst", bufs=1) as pst, \
                         tc.tile_pool(name="psts", bufs=1,
                                      space="PSUM") as psts:
                        sinv2 = pst.tile([P, NT], F32)
                        srow = pst.tile([1, N], F32)
                        nc.vector.tensor_scalar_mul(sinv2[:], sinv, 2.0)
                        stp = psts.tile([D, P], F32)
                        nc.tensor.transpose(stp[:], sinv2[:], ident[:])
                        st_sb = pst.tile([D, P], F32)
                        nc.vector.tensor_copy(st_sb[:], stp[:])
                        for t in range(D):
                            nc.sync.dma_start(
                                out=srow[0:1, t * P:(t + 1) * P],
                                in_=st_sb[t:t + 1, :])
                        nc.gpsimd.partition_broadcast(srep[:], srow[0:1, :])

                    # y1^T transpose emitter: per block m, 4 batch-PAIR
                    # transposes (each [128,128] block transposes two
                    # batches: rows 0:64 = even batch, 64:128 = odd),
                    # SBUF bounce, two strided DMAs into r0send.
                    def emit_tr(ptyp, y1tp, m):
                        pty = ptyp.tile([P, 4 * P], BF)
                        for bp_ in range(4):
                            nc.tensor.transpose(
                                pty[:, bp_ * P:(bp_ + 1) * P],
                                y1[:, m * BC + bp_ * P:
                                   m * BC + (bp_ + 1) * P],
                                identb[:])
                        y1t = y1tp.tile([P, 4 * P], BF)
                        nc.any.tensor_copy(y1t[:], pty[:])
                        dstv = r0send[m // 2, :, :].rearrange(
                            "c (b2 h n) -> c b2 h n", h=2, n=NSH)
                        for h in range(2):
                            nc.gpsimd.dma_start(
                                out=dstv[:, :, h:h + 1,
                                         (m % 2) * P:(m % 2) * P + P],
                                in_=y1t[h * C:(h + 1) * C, :]
                                .rearrange("c (b n) -> c b n",
                                           n=P).unsqueeze(2))

                    with tc.tile_pool(name="pty", bufs=2,
                                      space="PSUM") as ptyp, \
                         tc.tile_pool(name="y1tp", bufs=3) as y1tp:
                        # ==== phase C pass 2 (t = T1..T1+T2-1), with
                        # transposes of pass-1 blocks interleaved ====
                        with tc.tile_pool(name="ps2", bufs=1,
                                          space="PSUM") as ps2:
                            u2 = {t: ps2.tile([P, BC], F32,
                                              name=f"u2_{t}")
                                  for t in range(T1, T1 + T2)}
                            for m in range(NT):
                                for t in range(T1, T1 + T2):
                                    nc.tensor.matmul(
                                        u2[t][:],
                                        z[:, m * N + t * P:
                                          m * N + (t + 1) * P],
                                        xs[:, m * BC:(m + 1) * BC],
                                        start=(m == 0),
                                        stop=(m == NT - 1))
                                if m % 4 == 3:
                                    emit_tr(ptyp, y1tp, m // 4)
                            for t in range(T1, T1 + T2):
                                nc.vector.tensor_scalar_mul(
                                    y1[:, t * BC:(t + 1) * BC],
                                    u2[t][:], sinv[:, t:t + 1])

                        # ==== phase C pass 3 (t = T1+T2..15), with
                        # transposes of pass-2 blocks interleaved ====
                        with tc.tile_pool(name="ps3", bufs=1,
                                          space="PSUM") as ps3:
                            u3 = {t: ps3.tile([P, BC], F32,
                                              name=f"u3_{t}")
                                  for t in range(T1 + T2, NT)}
                            for m in range(NT):
                                for t in range(T1 + T2, NT):
                                    nc.tensor.matmul(
                                        u3[t][:],
                                        z[:, m * N + t * P:
                                          m * N + (t + 1) * P],
                                        xs[:, m * BC:(m + 1) * BC],
                                        start=(m == 0),
                                        stop=(m == NT - 1))
                                if m % 2 == 1 and m // 2 < T2:
                                    emit_tr(ptyp, y1tp, T1 + m // 2)
                            for t in range(T1 + T2, NT):
                                nc.vector.tensor_scalar_mul(
                                    y1[:, t * BC:(t + 1) * BC],
                                    u3[t][:], sinv[:, t:t + 1])
                            for m in range(T1 + T2, NT):
                                emit_tr(ptyp, y1tp, m)

                    # Exchange y1^T while D2 computes u2'.
                    nc.gpsimd.collective_compute(
                        "AllToAll", OP.bypass, replica_groups=GROUP,
                        ins=[r0send.opt()], outs=[r0recv.opt()])

                # ========== Phase D2: u2' = 2 (Z @ y1) / s ===========
                # u2t[(b,c), n'] = sum_n y1[n, bc] Z[n, n'] (Z symmetric)
                # comes out pre-transposed for rc1; scaled by 2/s via srep,
                # then DMA'd straight into r1send chunks.
                with tc.tile_pool(name="pu2", bufs=2, space="PSUM") as pu2, \
                     tc.tile_pool(name="y2p", bufs=2) as y2p:
                    for i in range(NCORES):
                        nc.sync.dma_start(
                            out=rr0[C:P, i * BSH * NSH:(i + 1) * BSH * NSH],
                            in_=r0recv[i, :, :])
                    for nq in range(4):
                        u2t = pu2.tile([P, 4 * 512], F32)
                        for m in range(NT):
                            for bc in range(4):
                                nc.tensor.matmul(
                                    u2t[:, bc * 512:(bc + 1) * 512],
                                    y1[:, m * BC + bc * P:
                                       m * BC + (bc + 1) * P],
                                    z[:, m * N + nq * 512:
                                      m * N + (nq + 1) * 512],
                                    start=(m == 0), stop=(m == NT - 1))
                        y2t = y2p.tile([P, 4 * 512], BF)
                        nc.vector.tensor_tensor(
                            out=y2t[:].rearrange("p (q n) -> p q n", q=4),
                            in0=u2t[:].rearrange("p (q n) -> p q n", q=4),
                            in1=srep[:, nq * 512:(nq + 1) * 512]
                            .unsqueeze(1).broadcast_to((P, 4, 512)),
                            op=OP.mult)
                        for bc in range(4):
                            for h in range(2):
                                bl = 2 * bc + h
                                for q2 in range(2):
                                    j = 2 * nq + q2
                                    nc.gpsimd.dma_start(
                                        out=r1send[j, 0:C, :].rearrange(
                                            "c (b n) -> c b n",
                                            n=NSH)[:, bl:bl + 1, :],
                                        in_=y2t[h * C:(h + 1) * C,
                                                bc * 512 + q2 * NSH:
                                                bc * 512 + (q2 + 1) * NSH]
                                        .unsqueeze(1))

                    nc.gpsimd.collective_compute(
                        "AllToAll", OP.bypass, replica_groups=GROUP,
                        ins=[r1send.opt()], outs=[r1recv.opt()])

            # ========= W build (overlaps AllToAll #2) + Phase F ==========
            # W0[ki, o*NSH + n] = W_eff[n, ki, o] (ki = k0 i | k1 i)
            # W1[i,  o*NSH + n] = W[n, k2, i, o]; row 64 = bias[n, o].
            with tc.tile_pool(name="pf", bufs=1) as pf:
                # Weight staging loads on the sync queue so they land
                # during D2, ahead of the W build.
                wk01sb = pf.tile([D, P * C], BF)
                nc.sync.dma_start(out=wk01sb[:], in_=wk01_in[:, :])
                wk2sb = pf.tile([D, (C + 1) * C], BF)
                nc.sync.dma_start(out=wk2sb[:], in_=wk2_in[:, :])
                etrsb = pf.tile([D, NSH], BF)
                nc.sync.dma_start(out=etrsb[:], in_=etr_in[:, :])
                w0 = pf.tile([P, C * NSH], BF)
                w1 = pf.tile([KR, C * NSH], BF)
                with tc.tile_pool(name="psW", bufs=4, space="PSUM") as psw, \
                     tc.tile_pool(name="psV", bufs=4, space="PSUM") as psv:
                    for op_ in range(C // 2):
                        pw = psw.tile([P, 2 * NSH], F32)
                        pv = psv.tile([KR, 2 * NSH], F32)
                        for h in range(2):
                            o = 2 * op_ + h
                            nc.tensor.matmul(
                                pw[:, h * NSH:(h + 1) * NSH],
                                wk01sb[:, o * P:(o + 1) * P],
                                etrsb[:], start=True, stop=True)
                            nc.tensor.matmul(
                                pv[:, h * NSH:(h + 1) * NSH],
                                wk2sb[:, o * (C + 1):(o + 1) * (C + 1)],
                                etrsb[:], start=True, stop=True)
                        nc.vector.tensor_copy(
                            w0[:, 2 * op_ * NSH:(2 * op_ + 2) * NSH],
                            pw[:])
                        nc.scalar.activation(
                            w1[:, 2 * op_ * NSH:(2 * op_ + 2) * NSH],
                            pv[:], AF.Copy)

                for i in range(NCORES):
                    nc.sync.dma_start(
                        out=rr1[:, i * BSH * NSH:(i + 1) * BSH * NSH],
                        in_=r1recv[i, :, :])

                rr0v = rr0[:, :].rearrange("p (b n) -> p b n", n=NSH)
                rr1v = rr1[:, :].rearrange("p (b n) -> p b n", n=NSH)
                w0v = w0[:, :].rearrange("p (o n) -> p o n", n=NSH)
                w1v = w1[:, :].rearrange("p (o n) -> p o n", n=NSH)
                NG = NSH // 16  # groups: 16 nodes = 2 halves x 8 slots

                def f_mm0(pout, g):
                    # Full group per slot (stop is a HW no-op); F1 then
                    # accumulates onto the finished values with start=F.
                    for idx in range(16):
                        n = 16 * g + idx
                        s, j = idx % 2, idx // 2
                        nc.tensor.matmul(
                            pout[C * s:C * (s + 1), j * C:(j + 1) * C],
                            rr0v[:, :, n], w0v[:, :, n],
                            start=True, stop=True,
                            tile_position=(0, C * s))

                def f_mm1(pout, g):
                    for idx in range(16):
                        n = 16 * g + idx
                        s, j = idx % 2, idx // 2
                        nc.tensor.matmul(
                            pout[C * s:C * (s + 1), j * C:(j + 1) * C],
                            rr1v[:, :, n], w1v[:, :, n],
                            start=False, stop=True,
                            skip_group_check=True,
                            tile_position=(0, C * s))

                def f_drain(outp, pout, g):
                    outsb = outp.tile([P, 512], F32)
                    nc.any.tensor_copy(outsb[:], pout[:])
                    for s in range(2):
                        nc.sync.dma_start(
                            out=out_d[:, g * 16:(g + 1) * 16, :]
                            .rearrange("b (j s) c -> s b j c",
                                       s=2)[s:s + 1],
                            in_=outsb[C * s:C * (s + 1), :]
                            .rearrange("b (j c) -> b j c", c=C))

                with tc.tile_pool(name="outp", bufs=2) as outp, \
                     tc.tile_pool(name="psFb", bufs=4,
                                  space="PSUM") as psfb:
                    for g in range(NG):
                        pout = psfb.tile([P, 512], F32)
                        for idx in range(16):
                            n = 16 * g + idx
                            s, j = idx % 2, idx // 2
                            osl = pout[C * s:C * (s + 1),
                                       j * C:(j + 1) * C]
                            nc.tensor.matmul(
                                osl, rr0v[:, :, n], w0v[:, :, n],
                                start=True, stop=False,
                                tile_position=(0, C * s))
                            nc.tensor.matmul(
                                osl, rr1v[:, :, n], w1v[:, :, n],
                                start=False, stop=True,
                                tile_position=(0, C * s))
                        f_drain(outp, pout, g)

    nc.compile()
    return nc


_CACHE = {}
_LOCK = threading.Lock()


def _get_program():
    with _LOCK:
        if "nc" not in _CACHE:
            _CACHE["nc"] = build_program()
        return _CACHE["nc"]


BF_NP = mybir.dt.np(BF)


def kernel(x, node_embeddings, weights_pool, bias_pool):
    x = np.ascontiguousarray(np.asarray(x, dtype=np.float32))
    emb = np.ascontiguousarray(np.asarray(node_embeddings, dtype=np.float32))
    wp = np.ascontiguousarray(np.asarray(weights_pool, dtype=np.float32))
    bp = np.ascontiguousarray(np.asarray(bias_pool, dtype=np.float32))

    # Host-side sharding / layout prep (bf16, PE-ready).
    et_h = np.ascontiguousarray(emb.T).astype(BF_NP)          # [D, N]
    # wk01[d, o*128 + k*64 + i] = Wp_eff[d, k, i, o], k in {0, 1}
    wk01_h = np.empty((D, C, 2, C), dtype=np.float32)
    wk01_h[:, :, 0, :] = (wp[:, 0] - wp[:, 2]).transpose(0, 2, 1)
    wk01_h[:, :, 1, :] = wp[:, 1].transpose(0, 2, 1)
    wk01_h = wk01_h.reshape(D, P * C).astype(BF_NP)
    # wk2[d, o*65 + i] = Wp[d, 2, i, o]; i=64 col = bp[d, o]
    wk2_h = np.empty((D, C, C + 1), dtype=np.float32)
    wk2_h[:, :, 0:C] = wp[:, 2].transpose(0, 2, 1)
    wk2_h[:, :, C] = bp
    wk2_h = wk2_h.reshape(D, (C + 1) * C).astype(BF_NP)

    nc = _get_program()
    core_ids = list(range(NCORES))
    in_maps = []
    for i in core_ids:
        xb = x[i * BSH:(i + 1) * BSH]                          # [8, N, C]
        xt_h = np.ascontiguousarray(
            xb.transpose(1, 0, 2)).reshape(NT, P, BC).astype(BF_NP)
        xn = x[:, i * NSH:(i + 1) * NSH, :]                    # [B, NSH, C]
        xnt_h = np.ascontiguousarray(
            xn.transpose(2, 0, 1)).reshape(C, B * NSH).astype(BF_NP)
        etr_h = np.ascontiguousarray(
            emb[i * NSH:(i + 1) * NSH].T).astype(BF_NP)        # [D, NSH]
        in_maps.append({"xt": xt_h, "xnt": xnt_h, "et": et_h,
                        "etr": etr_h, "wk01": wk01_h, "wk2": wk2_h})
    trace = os.environ.get("KERNEL_TRACE", "") == "1"
    res = run_bass_kernel_spmd(nc, in_maps, core_ids, trace=trace)
    if trace:
        kernel.last_exec_time_ns = res.exec_time_ns
        kernel.last_results = res
    out = np.concatenate([res.results[i]["out"] for i in core_ids], axis=1)
    return out


kernel.last_exec_time_ns = None

if __name__ == "__main__":
    rng = np.random.default_rng(0)
    ins = {
        "x": rng.standard_normal((B, N, C), dtype=np.float32),
        "node_embeddings": rng.standard_normal((N, D), dtype=np.float32),
        "weights_pool": (rng.standard_normal((D, K, C, C), dtype=np.float32)
                         * 0.1),
        "bias_pool": rng.standard_normal((D, C), dtype=np.float32) * 0.1,
    }
    out = kernel(**ins)
    print("out", out.shape, out.dtype, float(np.abs(out).mean()))


# revision 27
# speedup vs baseline: 2.4311x; 2.4311x over previous
"""AVWGCN (adaptive-embedding graph conv) Trainium2 Bass kernel.

Math (reference):
    A   = softmax(relu(E E^T), axis=1)            # [N, N], E: [N, D]
    T0  = I, T1 = A, T2 = 2 A A - I               # Chebyshev supports
    W   = einsum('nd,dkio->nkio', E, Wp)          # per-node weights
    b   = E @ bp                                  # per-node bias
    x_g = einsum('knm,bmc->bnkc', T, x)
    out = einsum('bnki,nkio->bno', x_g, W) + b

Restructuring (algebraically exact up to bf16 rounding):
  * Z := exp(relu(E E^T)) = max(exp(E E^T), 1) is SYMMETRIC; with row
    sums s, A = Z/s. Z stays RESIDENT IN SBUF (64KB/partition) — no
    DRAM round trip. Row sums come for free from the DVE max pass via
    accum_out.
  * y1 = A @ X, u2' = 2 A y1 (so y2 = u2' - X); the "- X" is folded
    into the weights: W_eff[k0] = Wp[k0] - Wp[k2] applied to x, Wp[k2]
    applied to u2'.
  * Final stage is node-parallel: y1^T and u2'^T (columns (b, n)) are
    exchanged via two AllToAll collectives so each core holds all 64
    batches for its 256 nodes. x^T for this core's nodes is provided
    directly as a per-core host-prepared input (no exchange needed).
    Per-node weights W[n] = sum_d E[n,d] Wp_eff[d] (+ bias row, paired
    with the ones row of R) are built on the PE while AllToAll #2
    flies; each node's output is two small accumulating matmuls.
  * Host pre-transposes/casts inputs per core (x batch-shard in
    node-major bf16, x node-shard channel-major bf16, E^T, weight
    pools in PE-ready layout) — device does zero layout work on x/E/W
    inputs.

Sharding: batch-parallel aggregation (8 cores x 8 batches),
node-parallel final stage (8 cores x 256 nodes) with AllToAll
redistribution. All matmul operands are bf16 (PSUM stays fp32).
"""

import os
import sys
import threading

sys.path.insert(0, "/opt/trn_rl_repo")

import numpy as np

import concourse.bass as bass  # noqa: E402
import concourse.mybir as mybir  # noqa: E402
from concourse import bacc  # noqa: E402
from concourse.tile import TileContext  # noqa: E402
from concourse.masks import make_identity  # noqa: E402
from concourse.bass_utils import run_bass_kernel_spmd  # noqa: E402

F32 = mybir.dt.float32
BF = mybir.dt.bfloat16
AF = mybir.ActivationFunctionType
OP = mybir.AluOpType

NCORES = 8
B = 64
BSH = B // NCORES          # 8 batches per core
N = 2048
C = 64                     # C_IN == C_OUT
D = 16                     # embedding dim
K = 3                      # Chebyshev order
P = 128                    # partitions
NT = N // P                # 16 node blocks
BC = BSH * C               # 512 = per-core (b, c) width
KI = K * C                 # 192 contraction for the final stage
NSH = N // NCORES          # 256 nodes per core in the final stage
KR = KI - P + 1            # 65: rows of the second R tile (y2 + ones)
CH = NCORES * BSH * NSH // N  # unused; keep linter quiet

T1 = 4                     # pass-1 y1 blocks (PSUM: 4 u1 banks + 4 zt)
T2 = 6                     # pass-2 y1 blocks (+2 banks for transposes)
T3 = 6                     # pass-3 y1 blocks (+2 banks for transposes)


def build_program():
    nc = bacc.Bacc("TRN2", target_bir_lowering=False, debug=False,
                   num_devices=NCORES)

    # Host-prepared per-core inputs (bf16, PE-ready layouts).
    xt = nc.dram_tensor("xt", [NT, P, BC], BF, kind="ExternalInput")
    xnt = nc.dram_tensor("xnt", [C, B * NSH], BF, kind="ExternalInput")
    et_in = nc.dram_tensor("et", [D, N], BF, kind="ExternalInput")
    etr_in = nc.dram_tensor("etr", [D, NSH], BF, kind="ExternalInput")
    wk01_in = nc.dram_tensor("wk01", [D, P * C], BF, kind="ExternalInput")
    wk2_in = nc.dram_tensor("wk2", [D, (C + 1) * C], BF,
                            kind="ExternalInput")
    # node-sharded output: this core's NSH nodes, all B batches
    out_d = nc.dram_tensor("out", [B, NSH, C], F32, kind="ExternalOutput")
    GROUP = [list(range(NCORES))]

    with TileContext(nc) as tc:
        with tc.tile_pool(name="persist", bufs=1) as pp, \
             tc.tile_pool(name="ccd", bufs=1, space="DRAM") as ccd:
            ident = pp.tile([P, P], F32)
            make_identity(nc, ident[:])
            identb = pp.tile([P, P], BF)
            make_identity(nc, identb[:])

            # DRAM bounce tensors for the collectives.
            r0send = ccd.tile([NCORES, C, BSH * NSH], BF)
            r0recv = ccd.tile([NCORES, C, BSH * NSH], BF)
            r1send = ccd.tile([NCORES, KR, BSH * NSH], BF)
            r1recv = ccd.tile([NCORES, KR, BSH * NSH], BF)

            # s bookkeeping: per-chunk partial sums, then s and 1/s.
            s_parts = pp.tile([P, NT * 2], F32)
            s_all = pp.tile([P, 2 * NT], F32)
            s_sb = s_all[:, 0:NT]
            sinv = s_all[:, NT:2 * NT]
            srep = pp.tile([P, N], F32)   # 2/s[n] replicated on all rows

            ones_r = pp.tile([1, BSH * NSH], BF)
            nc.vector.memset(ones_r[:], 1.0)

            # rr0/rr1: R columns for ALL 64 batches x my NSH nodes,
            # cols = (b_global, n_local) = (src*8 + b_local, n).
            rr0 = pp.tile([P, B * NSH], BF)
            rr1 = pp.tile([KR, B * NSH], BF)
            # x^T rows of R0 come straight from the host (this core's
            # node shard) — off the critical path, no exchange.
            for bg in range(8):
                nc.gpsimd.dma_start(
                    out=rr0[0:C, bg * BSH * NSH:(bg + 1) * BSH * NSH],
                    in_=xnt[:, bg * BSH * NSH:(bg + 1) * BSH * NSH])
            # ones row of r1send is constant
            for j in range(NCORES):
                nc.gpsimd.dma_start(out=r1send[j, C:C + 1, :],
                                    in_=ones_r[:])

            with tc.tile_pool(name="pz", bufs=1) as pz:
                z = pz.tile([P, NT * N], BF)    # Z resident: 64KB/part
                y1 = pz.tile([P, NT * BC], BF)

                with tc.tile_pool(name="p1", bufs=1) as p1:
                    xs = p1.tile([P, NT * BC], BF)
                    etsb = p1.tile([D, N], BF)
                    nc.sync.dma_start(out=etsb[:], in_=et_in[:, :])
                    for m in range(NT):
                        nc.sync.dma_start(out=xs[:, m * BC:(m + 1) * BC],
                                          in_=xt[m, :, :])

                    # ==== fused phase B + C, split Z production ====
                    # Z chunk jj covers columns [jj*1024, (jj+1)*1024).
                    # y1 blocks t=0..7 read only chunk 0, so chunk 1's
                    # production rides pass 2 with no consumer waiting.
                    # Per (m, jj): PE emits 2x512-col matmuls (contraction
                    # D=16), scalar exps to SBUF, DVE maxes in place and
                    # row-sum-reduces off-chain. u accumulation drains RAW
                    # (unscaled) bf16; the 1/s scale is applied in place
                    # during pass 3 once s is complete.
                    def z_chunk(psz, m, jj):
                        zt = psz.tile([P, 1024], F32)
                        for q in range(2):
                            j = 2 * jj + q
                            nc.tensor.matmul(
                                zt[:, q * 512:(q + 1) * 512],
                                etsb[:, m * P:(m + 1) * P],
                                etsb[:, j * 512:(j + 1) * 512],
                                start=True, stop=True)
                        zc = z[:, m * N + jj * 1024:
                               m * N + (jj + 1) * 1024]
                        nc.scalar.activation(zc, zt[:], AF.Exp)
                        nc.vector.tensor_scalar_max(zc, zc, 1.0)
                        nc.vector.reduce_sum(
                            out=s_parts[:, jj * NT + m:jj * NT + m + 1],
                            in_=zc, axis=mybir.AxisListType.X)

                    def u_acc(u, ts, m):
                        for t in ts:
                            nc.tensor.matmul(
                                u[t][:],
                                z[:, m * N + t * P:m * N + (t + 1) * P],
                                xs[:, m * BC:(m + 1) * BC],
                                start=(m == 0), stop=(m == NT - 1))

                    with tc.tile_pool(name="psz", bufs=2,
                                      space="PSUM") as psz:
                        # -- pass 1: Z chunk 0 + y1 blocks t=0..3 --
                        with tc.tile_pool(name="ps1", bufs=1,
                                          space="PSUM") as ps1:
                            u1 = {t: ps1.tile([P, BC], F32,
                                              name=f"u1_{t}")
                                  for t in range(4)}
                            for m in range(NT):
                                z_chunk(psz, m, 0)
                                if m >= 2:
                                    u_acc(u1, range(4), m - 2)
                            u_acc(u1, range(4), NT - 2)
                            u_acc(u1, range(4), NT - 1)
                            for t in range(4):
                                nc.vector.tensor_copy(
                                    y1[:, t * BC:(t + 1) * BC], u1[t][:])

                        # -- pass 2: Z chunk 1 + y1 blocks t=4..7 --
                        with tc.tile_pool(name="ps2", bufs=1,
                                          space="PSUM") as ps2:
                            u2 = {t: ps2.tile([P, BC], F32,
                                              name=f"u2_{t}")
                                  for t in range(4, 8)}
                            for m in range(NT):
                                z_chunk(psz, m, 1)
                                u_acc(u2, range(4, 8), m)
                            for t in range(4, 8):
                                nc.vector.tensor_copy(
                                    y1[:, t * BC:(t + 1) * BC], u2[t][:])

                    # s = row sums of Z (sum of the two chunk partials),
                    # then 1/s.
                    nc.vector.tensor_tensor(
                        out=s_sb, in0=s_parts[:, 0:NT],
                        in1=s_parts[:, NT:2 * NT], op=OP.add)
                    nc.vector.reciprocal(sinv, s_sb)

                    # srep: 2/s[n] replicated on all rows (for u2' col
                    # scaling). Built during pass 2 (off PE critical path
                    # except one tiny transpose).
                    with tc.tile_pool(name="pst", bufs=1) as pst, \
                         tc.tile_pool(name="psts", bufs=1,
                                      space="PSUM") as psts:
                        sinv2 = pst.tile([P, NT], F32)
                        srow = pst.tile([1, N], F32)
                        nc.vector.tensor_scalar_mul(sinv2[:], sinv, 2.0)
                        stp = psts.tile([D, P], F32)
                        nc.tensor.transpose(stp[:], sinv2[:], ident[:])
                        st_sb = pst.tile([D, P], F32)
                        nc.vector.tensor_copy(st_sb[:], stp[:])
                        for t in range(D):
                            nc.sync.dma_start(
                                out=srow[0:1, t * P:(t + 1) * P],
                                in_=st_sb[t:t + 1, :])
                        nc.gpsimd.partition_broadcast(srep[:], srow[0:1, :])

                    # y1^T transpose emitter: per block m, 4 batch-PAIR
                    # transposes (each [128,128] block transposes two
                    # batches: rows 0:64 = even batch, 64:128 = odd),
                    # SBUF bounce, two strided DMAs into r0send.
                    def emit_tr(ptyp, y1tp, m):
                        pty = ptyp.tile([P, 4 * P], BF)
                        for bp_ in range(4):
                            nc.tensor.transpose(
                                pty[:, bp_ * P:(bp_ + 1) * P],
                                y1[:, m * BC + bp_ * P:
                                   m * BC + (bp_ + 1) * P],
                                identb[:])
                        y1t = y1tp.tile([P, 4 * P], BF)
                        nc.any.tensor_copy(y1t[:], pty[:])
                        dstv = r0send[m // 2, :, :].rearrange(
                            "c (b2 h n) -> c b2 h n", h=2, n=NSH)
                        for h in range(2):
                            nc.gpsimd.dma_start(
                                out=dstv[:, :, h:h + 1,
                                         (m % 2) * P:(m % 2) * P + P],
                                in_=y1t[h * C:(h + 1) * C, :]
                                .rearrange("c (b n) -> c b n",
                                           n=P).unsqueeze(2))

                    with tc.tile_pool(name="pty", bufs=2,
                                      space="PSUM") as ptyp, \
                         tc.tile_pool(name="y1tp", bufs=3) as y1tp:
                        # ==== pass 3a (t = 8..11): in-place scaling of
                        # y1 blocks 0..7 + their transposes interleave ====
                        with tc.tile_pool(name="ps3a", bufs=1,
                                          space="PSUM") as ps3a:
                            u3 = {t: ps3a.tile([P, BC], F32,
                                               name=f"u3_{t}")
                                  for t in range(8, 12)}
                            for m in range(NT):
                                u_acc(u3, range(8, 12), m)
                                if m % 2 == 1:
                                    blk = m // 2
                                    nc.vector.tensor_scalar_mul(
                                        y1[:, blk * BC:(blk + 1) * BC],
                                        y1[:, blk * BC:(blk + 1) * BC],
                                        sinv[:, blk:blk + 1])
                                    emit_tr(ptyp, y1tp, blk)
                            for t in range(8, 12):
                                nc.vector.tensor_scalar_mul(
                                    y1[:, t * BC:(t + 1) * BC],
                                    u3[t][:], sinv[:, t:t + 1])

                        # ==== pass 3b (t = 12..15) + transposes 8..15 ====
                        with tc.tile_pool(name="ps3b", bufs=1,
                                          space="PSUM") as ps3b:
                            u4 = {t: ps3b.tile([P, BC], F32,
                                               name=f"u4_{t}")
                                  for t in range(12, NT)}
                            for m in range(NT):
                                u_acc(u4, range(12, NT), m)
                                if m % 2 == 1 and m // 2 < 4:
                                    emit_tr(ptyp, y1tp, 8 + m // 2)
                            for t in range(12, NT):
                                nc.vector.tensor_scalar_mul(
                                    y1[:, t * BC:(t + 1) * BC],
                                    u4[t][:], sinv[:, t:t + 1])
                            for m in range(12, NT):
                                emit_tr(ptyp, y1tp, m)

                    # Exchange y1^T while D2 computes u2'.
                    nc.gpsimd.collective_compute(
                        "AllToAll", OP.bypass, replica_groups=GROUP,
                        ins=[r0send.opt()], outs=[r0recv.opt()])

                # ========== Phase D2: u2' = 2 (Z @ y1) / s ===========
                # u2t[(b,c), n'] = sum_n y1[n, bc] Z[n, n'] (Z symmetric)
                # comes out pre-transposed for rc1; scaled by 2/s via srep,
                # then DMA'd straight into r1send chunks.
                with tc.tile_pool(name="pu2", bufs=2, space="PSUM") as pu2, \
                     tc.tile_pool(name="y2p", bufs=2) as y2p:
                    for i in range(NCORES):
                        nc.sync.dma_start(
                            out=rr0[C:P, i * BSH * NSH:(i + 1) * BSH * NSH],
                            in_=r0recv[i, :, :])
                    for nq in range(4):
                        u2t = pu2.tile([P, 4 * 512], F32)
                        for m in range(NT):
                            for bc in range(4):
                                nc.tensor.matmul(
                                    u2t[:, bc * 512:(bc + 1) * 512],
                                    y1[:, m * BC + bc * P:
                                       m * BC + (bc + 1) * P],
                                    z[:, m * N + nq * 512:
                                      m * N + (nq + 1) * 512],
                                    start=(m == 0), stop=(m == NT - 1))
                        y2t = y2p.tile([P, 4 * 512], BF)
                        nc.vector.tensor_tensor(
                            out=y2t[:].rearrange("p (q n) -> p q n", q=4),
                            in0=u2t[:].rearrange("p (q n) -> p q n", q=4),
                            in1=srep[:, nq * 512:(nq + 1) * 512]
                            .unsqueeze(1).broadcast_to((P, 4, 512)),
                            op=OP.mult)
                        for bc in range(4):
                            for h in range(2):
                                bl = 2 * bc + h
                                for q2 in range(2):
                                    j = 2 * nq + q2
                                    nc.gpsimd.dma_start(
                                        out=r1send[j, 0:C, :].rearrange(
                                            "c (b n) -> c b n",
                                            n=NSH)[:, bl:bl + 1, :],
                                        in_=y2t[h * C:(h + 1) * C,
                                                bc * 512 + q2 * NSH:
                                                bc * 512 + (q2 + 1) * NSH]
                                        .unsqueeze(1))

                    nc.gpsimd.collective_compute(
                        "AllToAll", OP.bypass, replica_groups=GROUP,
                        ins=[r1send.opt()], outs=[r1recv.opt()])

            # ========= W build (overlaps AllToAll #2) + Phase F ==========
            # W0[ki, o*NSH + n] = W_eff[n, ki, o] (ki = k0 i | k1 i)
            # W1[i,  o*NSH + n] = W[n, k2, i, o]; row 64 = bias[n, o].
            with tc.tile_pool(name="pf", bufs=1) as pf:
                # Weight staging loads on the sync queue so they land
                # during D2, ahead of the W build.
                wk01sb = pf.tile([D, P * C], BF)
                nc.sync.dma_start(out=wk01sb[:], in_=wk01_in[:, :])
                wk2sb = pf.tile([D, (C + 1) * C], BF)
                nc.sync.dma_start(out=wk2sb[:], in_=wk2_in[:, :])
                etrsb = pf.tile([D, NSH], BF)
                nc.sync.dma_start(out=etrsb[:], in_=etr_in[:, :])
                w0 = pf.tile([P, C * NSH], BF)
                w1 = pf.tile([KR, C * NSH], BF)
                with tc.tile_pool(name="psW", bufs=4, space="PSUM") as psw, \
                     tc.tile_pool(name="psV", bufs=4, space="PSUM") as psv:
                    for op_ in range(C // 2):
                        pw = psw.tile([P, 2 * NSH], F32)
                        pv = psv.tile([KR, 2 * NSH], F32)
                        for h in range(2):
                            o = 2 * op_ + h
                            nc.tensor.matmul(
                                pw[:, h * NSH:(h + 1) * NSH],
                                wk01sb[:, o * P:(o + 1) * P],
                                etrsb[:], start=True, stop=True)
                            nc.tensor.matmul(
                                pv[:, h * NSH:(h + 1) * NSH],
                                wk2sb[:, o * (C + 1):(o + 1) * (C + 1)],
                                etrsb[:], start=True, stop=True)
                        nc.vector.tensor_copy(
                            w0[:, 2 * op_ * NSH:(2 * op_ + 2) * NSH],
                            pw[:])
                        nc.scalar.activation(
                            w1[:, 2 * op_ * NSH:(2 * op_ + 2) * NSH],
                            pv[:], AF.Copy)

                for i in range(NCORES):
                    nc.sync.dma_start(
                        out=rr1[:, i * BSH * NSH:(i + 1) * BSH * NSH],
                        in_=r1recv[i, :, :])

                rr0v = rr0[:, :].rearrange("p (b n) -> p b n", n=NSH)
                rr1v = rr1[:, :].rearrange("p (b n) -> p b n", n=NSH)
                w0v = w0[:, :].rearrange("p (o n) -> p o n", n=NSH)
                w1v = w1[:, :].rearrange("p (o n) -> p o n", n=NSH)
                NG = NSH // 16  # groups: 16 nodes = 2 halves x 8 slots

                def f_mm0(pout, g):
                    # Full group per slot (stop is a HW no-op); F1 then
                    # accumulates onto the finished values with start=F.
                    for idx in range(16):
                        n = 16 * g + idx
                        s, j = idx % 2, idx // 2
                        nc.tensor.matmul(
                            pout[C * s:C * (s + 1), j * C:(j + 1) * C],
                            rr0v[:, :, n], w0v[:, :, n],
                            start=True, stop=True,
                            tile_position=(0, C * s))

                def f_mm1(pout, g):
                    for idx in range(16):
                        n = 16 * g + idx
                        s, j = idx % 2, idx // 2
                        nc.tensor.matmul(
                            pout[C * s:C * (s + 1), j * C:(j + 1) * C],
                            rr1v[:, :, n], w1v[:, :, n],
                            start=False, stop=True,
                            skip_group_check=True,
                            tile_position=(0, C * s))

                def f_drain(outp, pout, g):
                    outsb = outp.tile([P, 512], F32)
                    nc.any.tensor_copy(outsb[:], pout[:])
                    for s in range(2):
                        nc.sync.dma_start(
                            out=out_d[:, g * 16:(g + 1) * 16, :]
                            .rearrange("b (j s) c -> s b j c",
                                       s=2)[s:s + 1],
                            in_=outsb[C * s:C * (s + 1), :]
                            .rearrange("b (j c) -> b j c", c=C))

                with tc.tile_pool(name="outp", bufs=2) as outp, \
                     tc.tile_pool(name="psFb", bufs=4,
                                  space="PSUM") as psfb:
                    for g in range(NG):
                        pout = psfb.tile([P, 512], F32)
                        for idx in range(16):
                            n = 16 * g + idx
                            s, j = idx % 2, idx // 2
                            osl = pout[C * s:C * (s + 1),
                                       j * C:(j + 1) * C]
                            nc.tensor.matmul(
                                osl, rr0v[:, :, n], w0v[:, :, n],
                                start=True, stop=False,
                                tile_position=(0, C * s))
                            nc.tensor.matmul(
                                osl, rr1v[:, :, n], w1v[:, :, n],
                                start=False, stop=True,
                                tile_position=(0, C * s))
                        f_drain(outp, pout, g)

    nc.compile()
    return nc


_CACHE = {}
_LOCK = threading.Lock()


def _get_program():
    with _LOCK:
        if "nc" not in _CACHE:
            _CACHE["nc"] = build_program()
        return _CACHE["nc"]


BF_NP = mybir.dt.np(BF)


def kernel(x, node_embeddings, weights_pool, bias_pool):
    x = np.ascontiguousarray(np.asarray(x, dtype=np.float32))
    emb = np.ascontiguousarray(np.asarray(node_embeddings, dtype=np.float32))
    wp = np.ascontiguousarray(np.asarray(weights_pool, dtype=np.float32))
    bp = np.ascontiguousarray(np.asarray(bias_pool, dtype=np.float32))

    # Host-side sharding / layout prep (bf16, PE-ready).
    et_h = np.ascontiguousarray(emb.T).astype(BF_NP)          # [D, N]
    # wk01[d, o*128 + k*64 + i] = Wp_eff[d, k, i, o], k in {0, 1}
    wk01_h = np.empty((D, C, 2, C), dtype=np.float32)
    wk01_h[:, :, 0, :] = (wp[:, 0] - wp[:, 2]).transpose(0, 2, 1)
    wk01_h[:, :, 1, :] = wp[:, 1].transpose(0, 2, 1)
    wk01_h = wk01_h.reshape(D, P * C).astype(BF_NP)
    # wk2[d, o*65 + i] = Wp[d, 2, i, o]; i=64 col = bp[d, o]
    wk2_h = np.empty((D, C, C + 1), dtype=np.float32)
    wk2_h[:, :, 0:C] = wp[:, 2].transpose(0, 2, 1)
    wk2_h[:, :, C] = bp
    wk2_h = wk2_h.reshape(D, (C + 1) * C).astype(BF_NP)

    nc = _get_program()
    core_ids = list(range(NCORES))
    in_maps = []
    for i in core_ids:
        xb = x[i * BSH:(i + 1) * BSH]                          # [8, N, C]
        xt_h = np.ascontiguousarray(
            xb.transpose(1, 0, 2)).reshape(NT, P, BC).astype(BF_NP)
        xn = x[:, i * NSH:(i + 1) * NSH, :]                    # [B, NSH, C]
        xnt_h = np.ascontiguousarray(
            xn.transpose(2, 0, 1)).reshape(C, B * NSH).astype(BF_NP)
        etr_h = np.ascontiguousarray(
            emb[i * NSH:(i + 1) * NSH].T).astype(BF_NP)        # [D, NSH]
        in_maps.append({"xt": xt_h, "xnt": xnt_h, "et": et_h,
                        "etr": etr_h, "wk01": wk01_h, "wk2": wk2_h})
    trace = os.environ.get("KERNEL_TRACE", "") == "1"
    res = run_bass_kernel_spmd(nc, in_maps, core_ids, trace=trace)
    if trace:
        kernel.last_exec_time_ns = res.exec_time_ns
        kernel.last_results = res
    out = np.concatenate([res.results[i]["out"] for i in core_ids], axis=1)
    return out


kernel.last_exec_time_ns = None

if __name__ == "__main__":
    rng = np.random.default_rng(0)
    ins = {
        "x": rng.standard_normal((B, N, C), dtype=np.float32),
        "node_embeddings": rng.standard_normal((N, D), dtype=np.float32),
        "weights_pool": (rng.standard_normal((D, K, C, C), dtype=np.float32)
                         * 0.1),
        "bias_pool": rng.standard_normal((D, C), dtype=np.float32) * 0.1,
    }
    out = kernel(**ins)
    print("out", out.shape, out.dtype, float(np.abs(out).mean()))
